# revision 2
# baseline (speedup 1.0000x reference)
import numpy as np
import ml_dtypes
from contextlib import ExitStack

import concourse.mybir as mybir
import concourse.bass as bass
import concourse.tile as tile
from concourse.bass_utils import run_bass_kernel_spmd

# nn_Predictor (moe_routing). L=6 streams, B=16384, D=512, NC=3992, 4 experts.
# Strategy: gate on host (fp64, as ever), then route: each token runs ONLY its
# selected expert. Host sorts tokens by expert and deals them round-robin
# across the 8 cores so every core gets identical per-expert segment sizes
# (SPMD program). Host also pre-transposes activations to [feat, tok] and
# folds W2_e @ dec_W1 into one 512x512 matrix per expert, so the device runs
# just three matmul stages: W1+relu, Me+sigmoid, dec2. All matmuls in bf16
# (same 1 cycle/col rate as fp32r, half the HBM traffic).
L, B, D, NCLS, NE = 6, 16384, 512, 3992, 4
NCORES = 8
KD = L * D                    # 3072 flat features
NKE = [12, 12, 24, 24]        # 128-row K chunks per expert
KLO = [0, 12, 0, 0]           # first K chunk per expert (front/back/all/all)
NCH = (NCLS + 511) // 512     # 8 output column chunks (last = 408)
EORDER = [0, 1, 2, 3]         # small-W1 experts first: their ~40us of compute
                              # covers the big e2/e3 weight prefetch

F32 = mybir.dt.float32
BF16 = mybir.dt.bfloat16
BF = ml_dtypes.bfloat16


def _build(segs):
    # segs in EORDER processing order; TP need not be 128-aligned
    TP = sum(segs)
    nc = bass.Bass("TRN2")

    xT = nc.dram_tensor("xT", [KD, TP], BF16, kind="ExternalInput")
    w1_in = [
        nc.dram_tensor(f"w1_{e}", [128, NKE[e], 512], BF16, kind="ExternalInput")
        for e in range(NE)
    ]
    me_in = nc.dram_tensor("me", [128, NE * 4, 512], BF16, kind="ExternalInput")
    dw2_in = nc.dram_tensor("dw2", [128, 4, NCLS], BF16, kind="ExternalInput")
    bc_in = nc.dram_tensor("bc", [128, NE * 8], F32, kind="ExternalInput")
    out = nc.dram_tensor("out", [TP, NCLS], BF16, kind="ExternalOutput")

    with tile.TileContext(nc) as tc, ExitStack() as ctx:
        singles = ctx.enter_context(tc.tile_pool(name="singles", bufs=1))
        # W1 streams through a 4-slot FIFO of 6-chunk groups: the slot queue
        # naturally throttles prefetch into need-order.
        w1gP = ctx.enter_context(tc.tile_pool(name="w1gP", bufs=4))
        xtP = ctx.enter_context(tc.tile_pool(name="xtP", bufs=3))
        hP = ctx.enter_context(tc.tile_pool(name="hP", bufs=8))
        otP = ctx.enter_context(tc.tile_pool(name="otP", bufs=3))

        # one shared pool: all 8 banks flow between the 4-high W1/Me
        # accumulators and the interleaved dec2 stage
        psP = ctx.enter_context(tc.tile_pool(name="psP", bufs=8, space="PSUM"))

        sig = [singles.tile([128, TP], BF16, name=f"sig{m}") for m in range(4)]
        bcsb = singles.tile([128, NE * 8], F32)
        gsrc = singles.tile([128, 8], BF16)   # written once e0 is in flight;
                                              # WAW-gates the big prefetches
        mesb = singles.tile([128, NE * 4, 512], BF16)
        dw2sb = singles.tile([128, 4, NCLS], BF16)

        def dec2_subtile(s):
            # flip back to natural layout, [tok, cls] bf16 out (db2 added on
            # host). PSUM->SBUF copies split 6 vector / 2 scalar so neither
            # engine gates PSUM release.
            sw = min(128, TP - s)
            ot = otP.tile([128, NCLS], BF16, name="ot")
            for n in range(NCH):
                nw = min(512, NCLS - n * 512)
                ps = psP.tile([128, 512], F32, tag="ps", name="dps")
                for kh in range(4):
                    nc.tensor.matmul(
                        ps[:sw, :nw],
                        sig[kh][:, s : s + sw],
                        dw2sb[:, kh, n * 512 : n * 512 + nw],
                        start=(kh == 0),
                        stop=(kh == 3),
                    )
                if n % 8 < 6:
                    nc.vector.tensor_copy(
                        out=ot[:sw, n * 512 : n * 512 + nw], in_=ps[:sw, :nw]
                    )
                else:
                    nc.scalar.activation(
                        ot[:sw, n * 512 : n * 512 + nw], ps[:sw, :nw],
                        mybir.ActivationFunctionType.Copy,
                    )
            nc.sync.dma_start(out=out[s : s + sw, :], in_=ot[:sw, :])

        TW = max(segs)
        pos = 0
        s_done = 0   # next dec2 subtile start not yet emitted
        for ei, e in enumerate(EORDER):
            if segs[ei] == 0:
                continue
            nk = NKE[e]
            sg = segs[ei]
            ngrp = nk // 6
            xt = xtP.tile([128, 24, TW], BF16, name="xt")
            xt_ap = lambda g0, g1: bass.AP(
                tensor=xT,
                offset=(KLO[e] + g0) * 128 * TP + pos,
                ap=[[TP, 128], [128 * TP, g1 - g0], [1, sg]],
            )
            w1g = [w1gP.tile([128, 6, 512], BF16, name="w1g") for _ in range(ngrp)]
            if ei == 0:
                # interleave first-needed pieces for the fastest PE start
                nc.sync.dma_start(out=w1g[0], in_=w1_in[e][:, 0:6, :])
                nc.sync.dma_start(out=xt[:, 0:6, :sg], in_=xt_ap(0, 6))
                nc.sync.dma_start(out=bcsb, in_=bc_in[:, :])
                nc.sync.dma_start(out=w1g[1], in_=w1_in[e][:, 6:12, :])
                nc.sync.dma_start(out=xt[:, 6:nk, :sg], in_=xt_ap(6, nk))
                nc.sync.dma_start(out=mesb, in_=me_in[:, :, :])
            else:
                if ei == 2:
                    # keep e2's big activation load out of the critical
                    # lead-in window: release it once e0 is in flight
                    nc.vector.tensor_copy(
                        out=xt[0:1, 0, 0:1], in_=gsrc[0:1, 0:1]
                    )
                nc.sync.dma_start(out=xt[:, :nk, :sg], in_=xt_ap(0, nk))
                for g in range(ngrp):
                    nc.sync.dma_start(
                        out=w1g[g], in_=w1_in[e][:, 6 * g : 6 * g + 6, :]
                    )
            if ei == 2:
                # bridge: cover e2's W1 arrival with two dec2 subtiles of
                # already-finished tokens (dw2 landed during e1)
                for _ in range(2):
                    if s_done + 128 <= pos:
                        dec2_subtile(s_done)
                        s_done += 128
            for t0 in range(0, sg, 512):
                tcw = min(512, sg - t0)
                ps4 = [
                    psP.tile([128, 512], F32, tag="ps", name="w1ps")
                    for _ in range(4)
                ]
                for g in range(ngrp):
                    for kj6 in range(6):
                        kj = 6 * g + kj6
                        for m in range(4):
                            nc.tensor.matmul(
                                ps4[m][:, :tcw],
                                w1g[g][:, kj6, m * 128 : (m + 1) * 128],
                                xt[:, kj, :tcw],
                                start=(kj == 0),
                                stop=(kj == nk - 1),
                            )
                hs = []
                for m in range(4):
                    h = hP.tile([128, 512], BF16, name="h")
                    nc.scalar.activation(
                        h[:, :tcw], ps4[m][:, :tcw],
                        mybir.ActivationFunctionType.Relu,
                        bias=bcsb[:, e * 4 + m : e * 4 + m + 1], scale=1.0,
                    )
                    hs.append(h)
                if ei == 0 and t0 == 0:
                    nc.vector.tensor_copy(
                        out=gsrc[0:1, 0:1], in_=hs[0][0:1, 0:1]
                    )
                if ei == 1 and t0 == 0:
                    # dec2 weights: released only once e1 computes, so the
                    # 4MB load cannot steal lead-in bandwidth; arrives well
                    # before the first interleaved dec2 subtile
                    nc.vector.tensor_copy(
                        out=dw2sb[0:1, 0, 0:1], in_=hs[0][0:1, 0:1]
                    )
                    nc.sync.dma_start(out=dw2sb, in_=dw2_in[:, :, :])
                zs4 = [
                    psP.tile([128, 512], F32, tag="ps", name="zps")
                    for _ in range(4)
                ]
                for kh in range(4):
                    for m2 in range(4):
                        nc.tensor.matmul(
                            zs4[m2][:, :tcw],
                            mesb[:, e * 4 + kh, m2 * 128 : (m2 + 1) * 128],
                            hs[kh][:, :tcw],
                            start=(kh == 0),
                            stop=(kh == 3),
                        )
                for m2 in range(4):
                    nc.scalar.activation(
                        sig[m2][:, pos + t0 : pos + t0 + tcw], zs4[m2][:, :tcw],
                        mybir.ActivationFunctionType.Sigmoid,
                        bias=bcsb[:, 16 + e * 4 + m2 : 16 + e * 4 + m2 + 1],
                        scale=1.0,
                    )
                # interleave dec2 for any now-complete 128-token subtiles
                if ei >= 2:
                    while s_done + 128 <= pos + t0 + tcw:
                        dec2_subtile(s_done)
                        s_done += 128
            pos += segs[ei]

        while s_done < TP:
            dec2_subtile(s_done)
            s_done += min(128, TP - s_done)

    import bass_rust

    bass_rust.generate_event_semaphores(nc)
    return nc


_NC_CACHE = {}


def _get_nc(segs):
    key = tuple(segs)
    if key not in _NC_CACHE:
        _NC_CACHE[key] = _build(key)
    return _NC_CACHE[key]


def _route(inputs):
    """Host gate + routing plan. Returns (segs in EORDER, idx, flat)."""
    f32 = np.float32
    x = np.asarray(inputs["fusion_hs"], f32)                  # [L, B, D]
    flat = np.transpose(x, (1, 0, 2)).reshape(B, KD)          # [B, 6D]
    logits = flat.astype(np.float64) @ np.asarray(
        inputs["gate_W"], f32
    ).astype(np.float64) + np.asarray(inputs["gate_b"], f32).astype(np.float64)
    am = np.argmax(logits, axis=1)
    idx = [np.where(am == e)[0] for e in range(NE)]
    segs = [(len(idx[e]) + NCORES - 1) // NCORES for e in EORDER]
    return segs, idx, flat


def _prep_inputs(inputs, segs, idx, flat):
    f32 = np.float32
    TP = sum(segs)

    w1_raw = [
        np.asarray(inputs["e0_W1"], f32),
        np.asarray(inputs["e1_W1"], f32),
        np.asarray(inputs["e2_W1"], f32),
        np.array(inputs["e3_W1"], f32, copy=True),
    ]
    w1_raw[3][: 3 * D] *= f32(np.asarray(inputs["e3_a"]).reshape(-1)[0])
    w1_raw[3][3 * D :] *= f32(np.asarray(inputs["e3_b"]).reshape(-1)[0])

    dw1 = np.asarray(inputs["dec_W1"], f32)
    db1 = np.asarray(inputs["dec_b1"], f32)

    common = {}
    bc_cols = np.empty((128, NE * 8), f32)
    b1_cols = bc_cols[:, : NE * 4]
    ce_cols = bc_cols[:, NE * 4 :]
    me_host = np.empty((128, NE * 4, 512), BF)
    for e in range(NE):
        b1e = np.asarray(inputs[f"e{e}_b1"], f32)
        b1_cols[:, e * 4 : (e + 1) * 4] = b1e.reshape(4, 128).T
        mee = np.asarray(inputs[f"e{e}_W2"], f32) @ dw1        # [512hid, 512h2]
        cee = np.asarray(inputs[f"e{e}_b2"], f32) @ dw1 + db1  # [512]
        ce_cols[:, e * 4 : (e + 1) * 4] = cee.reshape(4, 128).T
        me_host[:, e * 4 : (e + 1) * 4, :] = (
            mee.reshape(4, 128, 512).transpose(1, 0, 2).astype(BF)
        )
        common[f"w1_{e}"] = np.ascontiguousarray(
            w1_raw[e].astype(BF).reshape(NKE[e], 128, 512).transpose(1, 0, 2)
        )
    common["bc"] = bc_cols
    common["me"] = me_host
    common["dw2"] = np.ascontiguousarray(
        np.asarray(inputs["dec_W2"], f32)
        .astype(BF)
        .reshape(4, 128, NCLS)
        .transpose(1, 0, 2)
    )

    flatT = np.ascontiguousarray(flat.T.astype(BF))            # [3072, B]
    in_maps = []
    for c in range(NCORES):
        perm = np.empty(TP, np.int64)
        p = 0
        for ei, e in enumerate(EORDER):
            ids = idx[e][c::NCORES]
            perm[p : p + len(ids)] = ids
            if segs[ei] > len(ids):
                perm[p + len(ids) : p + segs[ei]] = ids[-1] if len(ids) else 0
            p += segs[ei]
        m = dict(common)
        m["xT"] = np.ascontiguousarray(flatT[:, perm])
        in_maps.append(m)
    return in_maps


def _finish(inputs, segs, idx, results):
    f32 = np.float32
    db2 = np.asarray(inputs["dec_b2"], f32)
    out_full = np.empty((B, NCLS), f32)
    for c in range(NCORES):
        res = np.asarray(results[c]["out"]).astype(f32)        # [TP, NCLS]
        p = 0
        for ei, e in enumerate(EORDER):
            ids = idx[e][c::NCORES]
            out_full[ids] = res[p : p + len(ids)]
            p += segs[ei]
    out_full += db2
    return out_full


def _run(inputs, trace=False, tmpdir=None):
    segs, idx, flat = _route(inputs)
    nc = _get_nc(segs)
    in_maps = _prep_inputs(inputs, segs, idx, flat)
    res = run_bass_kernel_spmd(
        nc, in_maps, core_ids=list(range(NCORES)), trace=trace, tmpdir=tmpdir
    )
    out = _finish(inputs, segs, idx, res.results)
    return out, res


def kernel(**inputs):
    out, _ = _run(inputs)
    return out
